# revision 33
# baseline (speedup 1.0000x reference)
"""Trainium2 Bass kernel for LocalCrossCorrelationWithSmoothnessLoss.

Full inputs in, full output out. Pure data-parallel over batch (B=8 -> 8
NeuronCores); each core computes partial sums for its image; the host
combines them into the three scalar losses.

Per-core pipeline (one 1024x1024 image pair + two flow channels):
  premaps   I,J cast to bf16 (GPSIMD); I^2,J^2 (ACT); I*J (DVE) -> 5 bf16
            maps resident in SBUF.
  stage 1   fused transpose + H-direction 9-tap box conv on the PE:
            stationary = 128x128 premap block, moving = banded H matrix
            (81-scaled for the product maps)  ->  PSUM [w, h] fp32.
            This replaces the baseline's separate transpose pass.
  T-copy    PSUM -> SBUF bf16 (DVE/ACT alternating), chunked at stride
            120 along w with the 4-wide halo baked into the chunking.
  stage 2   W-direction box conv: stationary = banded W matrix, moving =
            T chunk -> PSUM [w_out, h] fp32.
  combine   crossN = 81S_IJ - S_I*S_J, IvarN = 81S_II - S_I^2,
            JvarN = 81S_JJ - S_J^2, cc = crossN^2 * exp(-ln(IvarN*JvarN))
            read directly from PSUM, bf16 intermediates (ln in fp32),
            split across DVE/ACT/GPSIMD, accumulated per-partition.
  smooth    sum(s^2) (ACT accum), lag products (DVE STT accum; row shift
            via SBUF->SBUF DMA). Edge corrections on the host.

Output per core: 68 partial sums. Host assembles the losses in float64.
"""
import sys
import numpy as np

sys.path.insert(0, "/opt/trn_rl_repo")

import ml_dtypes
import bass_rust
import concourse.bass as bass
import concourse.tile as tile
from concourse import mybir
from concourse import bass_utils
from concourse import tile_utils

F32 = mybir.dt.float32
BF16 = mybir.dt.bfloat16
ALU = mybir.AluOpType
ACTF = mybir.ActivationFunctionType

H = 1024
W = 1024
PAD = 4
ALPHA = 0.01
STRIDE = 120
NB = 8            # h blocks of 128

# W-chunk table: (out_lo, out_n, in_lo, in_n)
WCHUNKS = []
for _c in range((W + STRIDE - 1) // STRIDE):
    _olo = STRIDE * _c
    _on = min(STRIDE, W - _olo)
    _ilo = max(0, _olo - PAD)
    _ihi = min(W, _olo + _on + PAD)
    WCHUNKS.append((_olo, _on, _ilo, _ihi - _ilo))
NWC = len(WCHUNKS)

tile_utils.max_sbuf_usage = 206 * 1024

_nc_cache = {}


def _legalize_waits(nc, max_waits=1):
    """walrus accepts only one sync-wait per instruction; split extras
    onto same-engine NoOps placed just before."""
    ctr = 0
    for f in nc.m.functions:
        for bb in f.blocks:
            insts = bb.instructions
            i = 0
            while i < len(insts):
                ins = insts[i]
                si = ins.sync_info
                if si is None:
                    i += 1
                    continue
                w = list(si.on_wait)
                if len(w) <= max_waits:
                    i += 1
                    continue
                extra, keep = w[:-max_waits], w[-max_waits:]
                nops = []
                for j in range(0, len(extra), max_waits):
                    chunk = extra[j:j + max_waits]
                    nop = mybir.InstNoOp(name=f"I-wsplit-{ctr}", ins=[], outs=[])
                    ctr += 1
                    nop.engine = ins.engine
                    nop.sync_info = bass_rust.SyncInfo(on_wait=chunk, on_update=[])
                    nops.append(nop)
                ins.sync_info = bass_rust.SyncInfo(on_wait=keep,
                                                  on_update=list(si.on_update))
                insts[i:i] = nops
                i += len(nops) + 1


def _make_host_consts():
    """bandh [128, 272] bf16 (unscaled | x81), bandw [128, 256] bf16
    (chunk-0 variant | interior variant)."""
    bh = np.zeros((128, 136), dtype=np.float32)
    h = np.arange(128)[:, None]
    j = np.arange(136)[None, :]
    bh[(h >= j - 8) & (h <= j)] = 1.0
    bandh = np.concatenate([bh, bh * 81.0], axis=1).astype(ml_dtypes.bfloat16)

    r = np.arange(128)[:, None]
    q = np.arange(128)[None, :]
    b0 = (np.abs(r - q) <= PAD).astype(np.float32)          # chunk 0
    b1 = ((r >= q) & (r <= q + 8)).astype(np.float32)       # interior
    bandw = np.concatenate([b0, b1], axis=1).astype(ml_dtypes.bfloat16)
    return {"bandh": bandh, "bandw": bandw}


def _fused_mm_list(hh):
    """MM descriptors for one psum half tile (h_out in [512*hh, 512*hh+511]).
    Returns list of (h_block, j_lo, j_n, psum_col)."""
    mms = []
    for b in range(4 * hh, 4 * hh + 4):
        base = 128 * b - 4
        lo = max(512 * hh, base)
        hi = min(512 * hh + 511, base + 135)
        mms.append((b, lo - base, hi - lo + 1, lo - 512 * hh))
    if hh == 1:
        b = 4 * hh - 1
        base = 128 * b - 4
        mms.append((b, 512 * hh - base, 4, 0))
    if hh == 0:
        b = 4
        base = 128 * b - 4
        mms.append((b, 0, 4, base - 512 * hh))
    return mms


def _build(nc):
    I_d = nc.dram_tensor("I", [H, W], F32, kind="ExternalInput").ap()
    J_d = nc.dram_tensor("J", [H, W], F32, kind="ExternalInput").ap()
    s0_d = nc.dram_tensor("s0", [H, W], F32, kind="ExternalInput").ap()
    s1_d = nc.dram_tensor("s1", [H, W], F32, kind="ExternalInput").ap()
    bandh_d = nc.dram_tensor("bandh", [128, 272], BF16,
                             kind="ExternalInput").ap()
    bandw_d = nc.dram_tensor("bandw", [128, 256], BF16,
                             kind="ExternalInput").ap()
    part_d = nc.dram_tensor("partials", [128, 68], F32,
                            kind="ExternalOutput").ap()

    MAPS = ("si", "sj", "sij", "sii", "sjj")

    from contextlib import ExitStack
    with tile.TileContext(nc) as tc, ExitStack() as ctx:
        consts = ctx.enter_context(tc.tile_pool(name="consts", bufs=1))
        inp = ctx.enter_context(tc.tile_pool(name="inp", bufs=2))
        pmap = ctx.enter_context(tc.tile_pool(name="pmap", bufs=1))
        tmap = ctx.enter_context(tc.tile_pool(name="tmap", bufs=2))
        ctmp = ctx.enter_context(tc.tile_pool(name="ctmp", bufs=2))
        spool = ctx.enter_context(tc.tile_pool(name="spool", bufs=2))
        jpool = ctx.enter_context(tc.tile_pool(name="jpool", bufs=4))
        accp = ctx.enter_context(tc.tile_pool(name="accp", bufs=1))
        psA = ctx.enter_context(tc.tile_pool(name="psA", bufs=4, space="PSUM"))
        ps2 = ctx.enter_context(tc.tile_pool(name="ps2", bufs=1, space="PSUM"))

        bandh_t = consts.tile([128, 272], BF16)
        bandw_t = consts.tile([128, 256], BF16)
        nc.sync.dma_start(bandh_t[:], bandh_d)
        nc.sync.dma_start(bandw_t[:], bandw_d)

        # accumulators: accum_out OVERWRITES, so every accumulating
        # instruction gets its own column; host sums the groups.
        # cols 0-17: cc per (wchunk,half); 18-33: lag_w; 34-49: lag_h;
        # 52-67: s^2  (50-51 unused; layout matches host assembly)
        acc = accp.tile([128, 68], F32)
        nc.vector.memset(acc[:], 0.0)

        # ---------------- premaps: 5 bf16 maps per h-block ---------------
        pm = {}
        smooth_jobs = []
        for ch_i, s_d in enumerate((s0_d, s1_d)):
            for t in range(8):
                smooth_jobs.append((ch_i, t, s_d))

        def emit_smooth(job):
            ch_i, t, s_d = job
            st = spool.tile([128, W], F32, tag="s_in")
            eng_d = nc.sync if t % 2 == 0 else nc.scalar
            eng_d.dma_start(st[:], s_d[128 * t:128 * (t + 1), :])
            # sum s^2 (output is junk; only the accumulator matters;
            # bf16 junk keeps the DVE STTs in the fast mode)
            s2o = jpool.tile([128, W], BF16, tag="junk")
            nc.scalar.activation(s2o[:], st[:], ACTF.Square,
                                 accum_out=acc[:, 52 + ch_i * 8 + t:
                                               53 + ch_i * 8 + t])
            # lag_w: s[w]*s[w+1]
            lw = jpool.tile([128, W], BF16, tag="junk")
            nc.vector.scalar_tensor_tensor(
                out=lw[:, 0:W - 1], in0=st[:, 1:W], scalar=1.0,
                in1=st[:, 0:W - 1], op0=ALU.mult, op1=ALU.mult,
                accum_out=acc[:, 18 + ch_i * 8 + t:19 + ch_i * 8 + t])
            # lag_h: row-shifted copy loaded straight from DRAM (row t*128+1
            # onward), so s[h]*s[h+1] covers tile boundaries too
            nsh = 128 if t < 7 else 127
            sh = spool.tile([128, W], F32, tag="sh")
            eng_d2 = nc.scalar if t % 2 == 0 else nc.sync
            eng_d2.dma_start(sh[0:nsh, :],
                             s_d[128 * t + 1:128 * t + 1 + nsh, :])
            lh = jpool.tile([128, W], BF16, tag="junk")
            nc.vector.scalar_tensor_tensor(
                out=lh[0:nsh, :], in0=sh[0:nsh, :], scalar=1.0,
                in1=st[0:nsh, :], op0=ALU.mult, op1=ALU.mult,
                accum_out=acc[0:nsh, 34 + ch_i * 8 + t:
                              35 + ch_i * 8 + t])

        def emit_products(b):
            for name in MAPS:
                pm[(name, b)] = pmap.tile([128, W], BF16,
                                          tag=f"pm_{name}_{b}",
                                          name=f"pm_{name}_{b}")
            # I/J arrive as bf16 casting DMAs (SWDGE); all products derive
            # from the bf16 copies, so no fp32 staging loads at all
            nc.gpsimd.dma_start(pm[("si", b)][:],
                                I_d[128 * b:128 * (b + 1), :])
            nc.gpsimd.dma_start(pm[("sj", b)][:],
                                J_d[128 * b:128 * (b + 1), :])
            nc.scalar.square(pm[("sii", b)][:], pm[("si", b)][:])
            nc.scalar.square(pm[("sjj", b)][:], pm[("sj", b)][:])
            nc.vector.tensor_tensor(out=pm[("sij", b)][:],
                                    in0=pm[("si", b)][:],
                                    in1=pm[("sj", b)][:], op=ALU.mult)

        # half 0 of the image needs only h-blocks 0-4; emit those, start
        # the hh=0 sweep, and fold blocks 5-7 + smoothness into the sweep.
        for b in range(5):
            emit_products(b)

        # ------------- per (hh, w-chunk): fused H-conv+transpose, -------
        # ------------- W-conv, combine ----------------------------------
        for hh in range(2):
            for c, (olo, on, ilo, inn) in enumerate(WCHUNKS):
                if hh == 0 and c < 3:
                    emit_products(5 + c)       # blocks 5-7 ride the sweep
                if smooth_jobs:
                    emit_smooth(smooth_jobs.pop(0))
                t_tiles = {}
                for mi, name in enumerate(MAPS):
                    scaled = mi >= 2
                    bh_off = 136 if scaled else 0
                    tt = tmap.tile([128, 512], BF16, tag=f"T_{name}_{hh}")
                    t_tiles[name] = tt
                    pT = psA.tile([128, 512], F32, tag="psA")
                    mms = _fused_mm_list(hh)
                    for k, (b, jlo, jn, pcol) in enumerate(mms):
                        nc.tensor.matmul(
                            pT[0:inn, pcol:pcol + jn],
                            pm[(name, b)][:, ilo:ilo + inn],
                            bandh_t[:, bh_off + jlo:bh_off + jlo + jn],
                            start=(k == 0), stop=(k == len(mms) - 1),
                            skip_group_check=True,
                        )
                    if (mi + hh + c) % 3 == 0:
                        nc.vector.tensor_copy(tt[0:inn, :], pT[0:inn, :])
                    else:
                        nc.scalar.copy(tt[0:inn, :], pT[0:inn, :])

                bw_off = 0 if c == 0 else 128
                p2 = {}
                for name in MAPS:
                    if name == "sij":
                        # read-once map rides the fused-stage psum ring,
                        # freeing a bank to deepen that ring to 4
                        p2[name] = psA.tile([128, 512], F32, tag="psA",
                                            name=f"p2_{name}_{c}_{hh}")
                    else:
                        p2[name] = ps2.tile([128, 512], F32, tag=f"p2_{name}",
                                            name=f"p2_{name}_{c}_{hh}")
                    nc.tensor.matmul(p2[name][:, :],
                                     bandw_t[0:inn, bw_off:bw_off + 128],
                                     t_tiles[name][0:inn, :],
                                     start=True, stop=True)

                n = on
                # combine; bf16 intermediates, ln in fp32. The last chunks'
                # chains are tail-latency-critical: run their slow GPSIMD
                # ops on the (faster) DVE instead.
                eng_tt = nc.vector if (hh == 1 and c >= 7) else nc.gpsimd
                si_sb = ctmp.tile([128, 512], BF16, tag="si_sb")
                nc.scalar.copy(si_sb[0:n, :], p2["si"][0:n, :])
                P = ctmp.tile([128, 512], BF16, tag="P")
                nc.vector.tensor_tensor(out=P[0:n, :], in0=si_sb[0:n, :],
                                        in1=p2["sj"][0:n, :], op=ALU.mult)
                crossN = ctmp.tile([128, 512], BF16, tag="crossN")
                nc.vector.tensor_tensor(out=crossN[0:n, :],
                                        in0=p2["sij"][0:n, :],
                                        in1=P[0:n, :], op=ALU.subtract)
                si2 = ctmp.tile([128, 512], BF16, tag="si2")
                eng_tt.tensor_tensor(out=si2[0:n, :], in0=si_sb[0:n, :],
                                     in1=si_sb[0:n, :], op=ALU.mult)
                IvarN = ctmp.tile([128, 512], BF16, tag="IvarN")
                nc.vector.tensor_tensor(out=IvarN[0:n, :],
                                        in0=p2["sii"][0:n, :],
                                        in1=si2[0:n, :], op=ALU.subtract)
                sj2 = ctmp.tile([128, 512], BF16, tag="sj2")
                nc.scalar.square(sj2[0:n, :], p2["sj"][0:n, :])
                JvarN = ctmp.tile([128, 512], BF16, tag="JvarN")
                nc.vector.tensor_tensor(out=JvarN[0:n, :],
                                        in0=p2["sjj"][0:n, :],
                                        in1=sj2[0:n, :], op=ALU.subtract)
                denom = ctmp.tile([128, 512], BF16, tag="denom")
                eng_tt.tensor_tensor(out=denom[0:n, :], in0=IvarN[0:n, :],
                                     in1=JvarN[0:n, :], op=ALU.mult)
                lnd = ctmp.tile([128, 512], F32, tag="lnd")
                nc.scalar.activation(lnd[0:n, :], denom[0:n, :], ACTF.Ln)
                recip = ctmp.tile([128, 512], BF16, tag="recip")
                nc.scalar.activation(recip[0:n, :], lnd[0:n, :], ACTF.Exp,
                                     scale=-1.0)
                cross2 = ctmp.tile([128, 512], BF16, tag="cross2")
                eng_tt.tensor_tensor(out=cross2[0:n, :],
                                     in0=crossN[0:n, :],
                                     in1=crossN[0:n, :], op=ALU.mult)
                ccj = ctmp.tile([128, 512], BF16, tag="ccj")
                nc.vector.scalar_tensor_tensor(
                    out=ccj[0:n, :], in0=cross2[0:n, :], scalar=1.0,
                    in1=recip[0:n, :], op0=ALU.mult, op1=ALU.mult,
                    accum_out=acc[0:n, c * 2 + hh:c * 2 + hh + 1])

        # final partition reduction happens on the host (float64)
        nc.sync.dma_start(part_d, acc[:])

    return


def _get_nc():
    if "nc" not in _nc_cache:
        nc = bass.Bass("TRN2", target_bir_lowering=False, debug=False)
        _build(nc)
        _legalize_waits(nc)
        _nc_cache["nc"] = nc
    return _nc_cache["nc"]


def _make_in_maps(I, J, s):
    B = I.shape[0]
    consts = _make_host_consts()
    in_maps = []
    for b in range(B):
        m = {
            "I": np.ascontiguousarray(I[b, 0]),
            "J": np.ascontiguousarray(J[b, 0]),
            "s0": np.ascontiguousarray(s[b, 0]),
            "s1": np.ascontiguousarray(s[b, 1]),
        }
        m.update(consts)
        in_maps.append(m)
    return in_maps


def kernel(I, J, s, sum_filt):
    B = I.shape[0]
    assert I.shape == (B, 1, H, W) and s.shape == (B, 2, H, W)
    nc = _get_nc()
    in_maps = _make_in_maps(I, J, s)
    res = bass_utils.run_bass_kernel_spmd(nc, in_maps,
                                          core_ids=list(range(B)))
    parts = np.stack([res.results[b]["partials"] for b in range(B)])
    parts = parts.astype(np.float64).sum(axis=1)   # reduce partition dim

    # host-side final assembly (float64)
    s64 = s.astype(np.float64)
    cc_sum = float(parts[:, 0:18].sum())
    lag_w = parts[:, 18:34].sum(axis=1)
    lag_h = parts[:, 34:52].sum(axis=1)
    s2 = parts[:, 52:68].sum(axis=1)

    # edge corrections per core (both channels folded together)
    e_w = (s64[:, :, :, 0] ** 2).sum(axis=(1, 2)) + \
          (s64[:, :, :, -1] ** 2).sum(axis=(1, 2))
    e_h = (s64[:, :, 0, :] ** 2).sum(axis=(1, 2)) + \
          (s64[:, :, -1, :] ** 2).sum(axis=(1, 2))

    sum_dx2 = (2.0 * s2 - e_w - 2.0 * lag_w).sum()
    sum_dy2 = (2.0 * s2 - e_h - 2.0 * lag_h).sum()
    cnt = B * 2 * H * (W - 1)

    ncc_loss = -cc_sum / (B * H * W)
    smooth = 0.5 * (sum_dx2 / cnt + sum_dy2 / cnt) * ALPHA
    total = ncc_loss + smooth
    return np.array([total, ncc_loss, smooth], dtype=np.float32)
